# revision 13
# baseline (speedup 1.0000x reference)
"""Block-sparse linear y = x @ W^T + bias on 8 NeuronCores — K=128 scheme.

Data-parallel over tokens (512 rows of x per core). All matmuls use the
full 128-partition contraction: the x slot image holds PAIRS of input
column-blocks (c_top on partitions 0-63, c_bot on 64-127, same free
offset). A matmul for output row r with lhsT [128, 64] computes
B_{r,c_top}^T x_top + B_{r,c_bot}^T x_bot in one 512-cycle stream —
two blocks per matmul when both cells exist (a host-side max-cover
chooses slot pairs; each column may appear in several slots, the slot
image is pre-duplicated on the host). Unpaired blocks ride in a slot
with the other half zeroed.

Output rows are processed 2 per PSUM bank (halves h=0/1), 32 chunks,
8-deep bank rotation. MMs alternate halves so the two 128x64 col-half
regions of the PE run concurrently. Drain: ACT adds bias while copying
PSUM to SBUF, DMA to yT.
"""

import numpy as np
import ml_dtypes
from collections import OrderedDict, defaultdict

N_TOK, IN_F, OUT_F, BS, NCORES = 4096, 4096, 4096, 64, 8
NL = N_TOK // NCORES            # tokens per core (512)
GRID = OUT_F // BS              # 64 block-rows / block-cols
NCHUNK = 32                     # psum generations, 2 rows each
COPIES = 3                      # max slots per input column block

_CACHE = {}


def _plan(row_idx, col_idx, copies=COPIES, slot_budget=None, deg_cap=8):
    K = int(row_idx.shape[0])
    cells = OrderedDict()
    for k in range(K):
        cells.setdefault((int(row_idx[k]), int(col_idx[k])), []).append(k)
    rows_of_col = defaultdict(set)
    for (r, c) in cells:
        rows_of_col[c].add(r)

    # ---- choose slots: greedy edge cover ----
    # either per-column degree cap (copies) or a total slot budget with a
    # loose per-column cap (same DMA bytes, duplication goes where it pays)
    W = {}
    for a in range(GRID):
        for b in range(a + 1, GRID):
            w = len(rows_of_col[a] & rows_of_col[b])
            if w:
                W[(a, b)] = w
    deg = defaultdict(int)
    covered = set()               # blocks (r,c) already paired
    slots = []                    # list of (c_top, c_bot)
    slot_pairs = []               # per slot: list of rows r paired here
    cap = deg_cap if slot_budget is not None else copies
    while slot_budget is None or len(slots) < slot_budget:
        best, bm = None, 0
        for (a, b), w in W.items():
            if deg[a] >= cap or deg[b] >= cap:
                continue
            m = sum(1 for r in rows_of_col[a] & rows_of_col[b]
                    if (r, a) not in covered and (r, b) not in covered)
            if m > bm:
                bm, best = m, (a, b)
        if best is None or bm == 0:
            break
        a, b = best
        deg[a] += 1
        deg[b] += 1
        rs = [r for r in rows_of_col[a] & rows_of_col[b]
              if (r, a) not in covered and (r, b) not in covered]
        slots.append((a, b))
        slot_pairs.append(sorted(rs))
        for r in rs:
            covered.add((r, a))
            covered.add((r, b))
    # every column must appear in >=1 slot (for leftover half-MMs)
    missing = [c for c in range(GRID) if deg[c] == 0]
    for i in range(0, len(missing), 2):
        a = missing[i]
        b = missing[i + 1] if i + 1 < len(missing) else missing[0]
        slots.append((a, b))
        slot_pairs.append([])
        deg[a] += 1
        deg[b] += 1

    slot_of_col = defaultdict(list)   # c -> [(slot, half)]
    for si, (a, b) in enumerate(slots):
        slot_of_col[a].append((si, 0))
        slot_of_col[b].append((si, 1))

    # ---- build MM list: (r, slot, top_block, bot_block) ----
    mm_of = defaultdict(dict)     # (r, slot) -> {0: c_top?, 1: c_bot?}
    for si, rs in enumerate(slot_pairs):
        a, b = slots[si]
        for r in rs:
            mm_of[(r, si)] = {0: a, 1: b}
    # leftovers: first co-locate PAIRS of a row's leftover blocks whose
    # columns form an existing slot (one full MM instead of two halves),
    # then place singles, preferring to join a partially-filled MM
    leftovers_by_row = defaultdict(list)
    for (r, c) in cells:
        if (r, c) not in covered:
            leftovers_by_row[r].append(c)
    slot_index = {}
    for si, (a, b) in enumerate(slots):
        slot_index.setdefault((a, b), si)
    n_half = 0
    for r in sorted(leftovers_by_row):
        cs = sorted(set(leftovers_by_row[r]))
        merged = True
        while merged and len(cs) >= 2:
            merged = False
            for c1 in cs:
                for c2 in cs:
                    if c1 == c2:
                        continue
                    si = slot_index.get((c1, c2))
                    if si is not None and (r, si) not in mm_of:
                        mm_of[(r, si)] = {0: c1, 1: c2}
                        cs.remove(c1)
                        cs.remove(c2)
                        merged = True
                        break
                if merged:
                    break
        for c in cs:
            placed = False
            for (si, half) in slot_of_col[c]:
                d = mm_of.get((r, si))
                if d is not None and half not in d:
                    d[half] = c
                    placed = True
                    break
            if not placed:
                si, half = min(slot_of_col[c],
                               key=lambda sh: len(mm_of.get((r, sh[0]), ())))
                d = mm_of.setdefault((r, si), {})
                if half in d:       # slot position taken for this row
                    for (sj, hj) in slot_of_col[c]:
                        dj = mm_of.setdefault((r, sj), {})
                        if hj not in dj:
                            dj[hj] = c
                            placed = True
                            break
                    assert placed, (r, c)
                else:
                    d[half] = c
            n_half += 1

    mms_of_row = defaultdict(list)    # r -> [(slot, {half: c})]
    for (r, si), d in sorted(mm_of.items()):
        mms_of_row[r].append((si, d))

    # ---- rows -> 32 chunks x 2 halves ----
    # pair rows ADJACENT in MM-count order: minimizes within-chunk half
    # imbalance (imbalanced halves force tail MMs to run solo on the PE)
    order = sorted(range(GRID), key=lambda r: -len(mms_of_row[r]))
    chunk_rows = [[order[2 * i], order[2 * i + 1]] for i in range(NCHUNK)]

    chunks = []
    for ci in range(NCHUNK):
        r0, r1 = chunk_rows[ci]
        q0 = [(r0, si, d) for (si, d) in mms_of_row[r0]]
        q1 = [(r1, si, d) for (si, d) in mms_of_row[r1]]
        ent = []
        i0 = i1 = 0
        while i0 < len(q0) or i1 < len(q1):
            if i0 < len(q0):
                ent.append((0, *q0[i0])); i0 += 1
            if i1 < len(q1):
                ent.append((1, *q1[i1])); i1 += 1
        first, last = {}, {}
        for i, (h, r, si, d) in enumerate(ent):
            first.setdefault(h, i)
            last[h] = i
        entries = []
        for i, (h, r, si, d) in enumerate(ent):
            entries.append(dict(h=h, r=r, slot=si, blocks=dict(d),
                                start=(first[h] == i), stop=(last[h] == i)))
        chunks.append(dict(rows=(r0, r1), entries=entries))

    # narrow chunks first: fewer distinct slots gate the first matmuls, so
    # the PE starts sooner after the leading x-slot DMA group lands
    chunks.sort(key=lambda ch: len(ch["entries"]))

    # renumber slots by first use so early chunks only need early DMA groups
    first_use = {}
    for ci, ch in enumerate(chunks):
        for e in ch["entries"]:
            first_use.setdefault(e["slot"], ci)
    new_order = sorted(range(len(slots)), key=lambda s: (first_use.get(s, 99), s))
    remap = {old: new for new, old in enumerate(new_order)}
    slots = [slots[old] for old in new_order]
    for ch in chunks:
        for e in ch["entries"]:
            e["slot"] = remap[e["slot"]]

    return dict(cells=cells, slots=slots, chunks=chunks, n_half=n_half)


def _build_images(plan, blocks, bias):
    cells = plan["cells"]
    summed = {}
    for key, ks in cells.items():
        if len(ks) == 1:
            summed[key] = np.asarray(blocks[ks[0]], np.float32)
        else:
            acc = blocks[ks[0]].astype(np.float32).copy()
            for k in ks[1:]:
                acc += blocks[k]
            summed[key] = acc

    slots = plan["slots"]
    widths = [len(ch["entries"]) for ch in plan["chunks"]]
    wtot = sum(widths)
    img = np.zeros((128, wtot * BS), np.float32)
    seg = []
    off = 0
    for wch, ch in zip(widths, plan["chunks"]):
        for j, e in enumerate(ch["entries"]):
            cols = slice((off + j) * BS, (off + j + 1) * BS)
            for half, c in e["blocks"].items():
                B = summed[(e["r"], c)]
                img[half * 64:(half + 1) * 64, cols] = B.T
            e["loc"] = j
        seg.append((off, wch))
        off += wch

    bias_img = np.zeros((128, NCHUNK), np.float32)
    for ci, ch in enumerate(plan["chunks"]):
        r0, r1 = ch["rows"]
        bias_img[0:64, ci] = bias[r0 * BS:(r0 + 1) * BS]
        bias_img[64:128, ci] = bias[r1 * BS:(r1 + 1) * BS]

    return img.astype(ml_dtypes.bfloat16), bias_img, seg


def _build_xslots(plan, x_core):
    """x_core: [NL, IN_F] fp32 -> slot image [128, S*NL] bf16."""
    slots = plan["slots"]
    S = len(slots)
    xT = x_core.T.astype(ml_dtypes.bfloat16)      # [IN_F, NL]
    ximg = np.empty((128, S * NL), ml_dtypes.bfloat16)
    for si, (a, b) in enumerate(slots):
        ximg[0:64, si * NL:(si + 1) * NL] = xT[a * BS:(a + 1) * BS]
        ximg[64:128, si * NL:(si + 1) * NL] = xT[b * BS:(b + 1) * BS]
    return ximg


def _split_excess_waits(nc, mybir, limit=1):
    n = 0
    for fn in nc.m.functions:
        for bb in fn.blocks:
            out = []
            for inst in bb.instructions:
                si = inst.sync_info
                if si is not None and si.on_wait and len(si.on_wait) > limit:
                    waits = list(si.on_wait)
                    ups = list(si.on_update)
                    for j, w in enumerate(waits[:-limit]):
                        nop = mybir.InstNoOp(name=f"{inst.name}-ws{j}", ins=[], outs=[])
                        nop.engine = inst.engine
                        nop.sync_info = mybir.SyncInfo(on_wait=[w], on_update=[])
                        out.append(nop)
                        n += 1
                    inst.sync_info = mybir.SyncInfo(on_wait=waits[-limit:], on_update=ups)
                out.append(inst)
            bb.instructions = out
    return n


def _thin_engine_sem_updates(nc, mybir, engines=("EngineType.PE",)):
    """Drop per-instruction +1 sem increments that no wait ever observes."""
    insts = []
    for fn in nc.m.functions:
        for bb in fn.blocks:
            insts.extend(bb.instructions)

    upd_insts = defaultdict(list)
    upd_ok = defaultdict(lambda: True)
    upd_engine = {}
    waited = defaultdict(set)
    wait_ok = defaultdict(lambda: True)
    for inst in insts:
        si = inst.sync_info
        if si is None:
            continue
        for u in si.on_update:
            if u.sync_type != "semaphore":
                continue
            if u.update_mode != "sem-inc" or u.update_value != 1:
                upd_ok[u.id] = False
            e = str(inst.engine)
            if u.id in upd_engine and upd_engine[u.id] != e:
                upd_ok[u.id] = False
            upd_engine[u.id] = e
            upd_insts[u.id].append(inst)
        for w in si.on_wait:
            if w.sync_type != "semaphore":
                continue
            if w.wait_mode != "sem-ge-imm" or w.wait_reg is not None:
                wait_ok[w.id] = False
            waited[w.id].add(w.wait_value)

    victims = [s for s, il in upd_insts.items()
               if upd_ok[s] and wait_ok[s] and upd_engine.get(s) in engines
               and len(il) > 8]
    for s in victims:
        il = upd_insts[s]
        W = sorted(v for v in waited.get(s, set()) if 1 <= v <= len(il))
        keep_ticks = set(W)
        rank = {v: i + 1 for i, v in enumerate(W)}
        if len(il) not in keep_ticks:
            keep_ticks.add(len(il))
            rank[len(il)] = len(W) + 1
        for t, inst in enumerate(il, start=1):
            si = inst.sync_info
            ups = [u for u in si.on_update
                   if not (u.sync_type == "semaphore" and u.id == s)]
            if t in keep_ticks:
                ups.append(mybir.SyncUpdate(
                    sync_type="semaphore", id=s, ant_name=f"thin{s}",
                    update_mode="sem-inc", update_value=1, update_reg=None))
            inst.sync_info = mybir.SyncInfo(on_wait=list(si.on_wait),
                                            on_update=ups)
        for inst in insts:
            si = inst.sync_info
            if si is None or not si.on_wait:
                continue
            changed = False
            ws = []
            for w in si.on_wait:
                if w.sync_type == "semaphore" and w.id == s:
                    nv = rank.get(w.wait_value)
                    if nv is None:
                        nv = sum(1 for v in rank if v <= w.wait_value)
                    ws.append(mybir.SyncWait(
                        sync_type="semaphore", id=s, ant_name=f"thin{s}",
                        wait_mode="sem-ge-imm", wait_value=nv, wait_reg=None))
                    changed = True
                else:
                    ws.append(w)
            if changed:
                inst.sync_info = mybir.SyncInfo(on_wait=ws,
                                                on_update=list(si.on_update))
    return victims


DRAIN = "alt"                   # "alt" | "dve" | "split"


def _build_bass(plan, wimg, seg, reps=1, do_mm=True, do_drain=True,
                do_out=True, drain=None):
    import concourse.bass as bass
    import concourse.mybir as mybir
    import concourse.tile as tile

    F32 = mybir.dt.float32
    BF16 = mybir.dt.bfloat16
    S = len(plan["slots"])
    NG = 16                     # x slot dma groups
    GW = ((S + NG - 1) // NG)   # slots per group
    wmax = max(s[1] for s in seg)

    nc = bass.Bass()
    xd = nc.declare_dram_parameter("xs", [128, S * NL], BF16, isOutput=False)
    imd = nc.declare_dram_parameter("img", [128, wimg * BS], BF16, isOutput=False)
    bd = nc.declare_dram_parameter("bias_img", [128, NCHUNK], F32,
                                   isOutput=False)
    yTd = nc.declare_dram_parameter("yT", [NCHUNK, 128, NL], F32,
                                    isOutput=True)

    with tile.TileContext(nc) as tc:
        with (
            tc.tile_pool(name="xs", bufs=1) as xs_pool,
            tc.tile_pool(name="blk", bufs=3) as blk_pool,
            tc.tile_pool(name="cst", bufs=1) as cst_pool,
            tc.tile_pool(name="stp", bufs=4) as st_pool,
            tc.tile_pool(name="ps", bufs=1, space="PSUM") as ps_pool,
        ):
            bias_t = cst_pool.tile([128, NCHUNK], F32, tag="bias")
            nc.sync.dma_start(out=bias_t[:], in_=bd[:])

            for rep in range(reps):
                xs_tiles = []
                for g in range(NG):
                    lo = g * GW
                    hi = min(S, lo + GW)
                    t = xs_pool.tile([128, (hi - lo) * NL], BF16, tag=f"xs{g}",
                                     name=f"xs{rep}_{g}")
                    xs_tiles.append(t)
                    nc.sync.dma_start(out=t[:], in_=xd[:, lo * NL:hi * NL])

                for ci, ch in enumerate(plan["chunks"]):
                    coff, wch = seg[ci]
                    blk_t = blk_pool.tile([128, wmax * BS], BF16, tag="blk",
                                          name=f"blk{ci}")
                    nc.sync.dma_start(out=blk_t[:, :wch * BS],
                                      in_=imd[:, coff * BS:(coff + wch) * BS])

                    if do_mm:
                        ps = ps_pool.tile([128, NL], F32, tag=f"bank{ci % 8}",
                                          name=f"ps{ci}")
                        for e in ch["entries"]:
                            si = e["slot"]
                            g, gm = si // GW, si % GW
                            lhsT = blk_t[:, e["loc"] * BS:(e["loc"] + 1) * BS]
                            rhs = xs_tiles[g][:, gm * NL:(gm + 1) * NL]
                            h = e["h"]
                            out = ps[h * 64:(h + 1) * 64, :]
                            nc.tensor.matmul(out, lhsT, rhs, start=e["start"],
                                             stop=e["stop"],
                                             tile_position=(0, h * 64))

                    if do_mm and do_drain:
                        mode = drain or DRAIN
                        st = st_pool.tile([128, NL], F32, tag="st",
                                          name=f"st{ci}")
                        if mode == "dve":
                            nc.vector.tensor_scalar_add(
                                st[:], ps[:], bias_t[:, ci:ci + 1])
                        elif mode == "split":
                            # DVE and ACT each drain half the bank columns
                            nc.vector.tensor_scalar_add(
                                st[:, :NL // 2], ps[:, :NL // 2],
                                bias_t[:, ci:ci + 1])
                            nc.scalar.activation(
                                st[:, NL // 2:], ps[:, NL // 2:],
                                mybir.ActivationFunctionType.Identity,
                                bias=bias_t[:, ci:ci + 1])
                        elif ci % 2 == 0:
                            nc.scalar.activation(
                                st[:], ps[:],
                                mybir.ActivationFunctionType.Identity,
                                bias=bias_t[:, ci:ci + 1])
                        else:
                            nc.vector.tensor_scalar_add(
                                st[:], ps[:], bias_t[:, ci:ci + 1])
                        if do_out:
                            # two partition-half DMAs: contiguous 128KB each,
                            # two queues, halves the end-of-kernel DMA tail
                            nc.sync.dma_start(out=yTd[ci][0:64], in_=st[0:64])
                            nc.sync.dma_start(out=yTd[ci][64:128],
                                              in_=st[64:128])

    _thin_engine_sem_updates(nc, mybir)
    _split_excess_waits(nc, mybir)
    return nc


def kernel(x, blocks, bias, row_idx, col_idx):
    from concourse.bass_utils import run_bass_kernel_spmd

    row_idx = np.asarray(row_idx)
    col_idx = np.asarray(col_idx)
    key = (row_idx.tobytes(), col_idx.tobytes())
    if key not in _CACHE:
        _CACHE[key] = [_plan(row_idx, col_idx, slot_budget=94), None]
    plan = _CACHE[key][0]

    img, bias_img, seg = _build_images(plan, np.asarray(blocks),
                                       np.asarray(bias, np.float32))
    if _CACHE[key][1] is None:
        _CACHE[key][1] = _build_bass(plan, img.shape[1] // BS, seg)
    nc = _CACHE[key][1]

    x = np.asarray(x)
    in_maps = []
    for i in range(NCORES):
        ximg = _build_xslots(plan, x[i * NL:(i + 1) * NL, :])
        in_maps.append({"xs": ximg, "img": img, "bias_img": bias_img})

    res = run_bass_kernel_spmd(nc, in_maps, list(range(NCORES))).results

    y = np.empty((N_TOK, OUT_F), np.float32)
    for i in range(NCORES):
        raw = res[i]["yT"]
        yl = y[i * NL:(i + 1) * NL]
        for ci, ch in enumerate(plan["chunks"]):
            r0, r1 = ch["rows"]
            yl[:, r0 * BS:(r0 + 1) * BS] = raw[ci, 0:64, :].T
            yl[:, r1 * BS:(r1 + 1) * BS] = raw[ci, 64:128, :].T
    return y
